# revision 10
# baseline (speedup 1.0000x reference)
"""Contrastive loss kernel for Trainium2, 8 NeuronCores (SPMD).

Math (matches the reference):
    z = concat(normalize(z_i), normalize(z_j))        # (2B, D) = (8192, 256)
    sim = (z @ z.T) / T
    positives[g] = sim[g, (g+B) mod 2B]               # (2B,)
    neg_max[g] = max_{j != g} sim[g, j]
    loss = mean(neg_max) - logsumexp(positives)       # scalar

Sharding: data-parallel over rows. Core k receives z rolled by -1024*k so its
band is always rows [0, 1024) of its local copy -> identical static program on
every core (diagonal / positive blocks land at fixed tile offsets).

v5 design (fp8 DoubleRow):
  - Loads truncate f32 -> bf16 in the DMA itself (strided read of the high
    2 bytes), halving HBM load traffic.
  - Norms: ACT Square with accum_out (fused sum of squares per tile),
    sqrt(n2/256) on ACT then DVE reciprocal gives inv16 = 16/||z||.
  - Quantize: Pool tensor_scalar_mul scales each row tile by inv16 and
    downcasts to fp8e4 (values ~16x cos components, well inside e4m3 range).
  - fp8 rows round-trip DRAM; the transpose DMA (2-byte granularity) moves
    uint16 PAIRS of adjacent-d fp8 values; the matmul reads the pair tile
    through a bitcast AP as [128, 2, cols]. Contraction maps (p, i) -> d =
    2p+i consistently for both operands, so a single DoubleRow matmul does
    the full K=256 contraction at 0.5 cycles/row.
  - Main loop is column-group-major: quad q over all 8 row blocks as one
    wave, started as soon as group q (2048 cols) is transposed. PSUM quads
    are consumed by alternating terminal reducers: DVE tensor_tensor_reduce
    (max of halves + max-reduce in one op) and Pool fold + DVE short reduce.
  - Preprocessing of group g+2 is emitted interleaved with wave g so every
    engine's program order matches the pipeline order.
Host: gather, undo the 256x fp8 scaling, /T, mean/LSE in float64.
"""

import numpy as np

TEMPERATURE = 0.1
B, D = 4096, 256
R = 2 * B                # 8192 total rows
NCORES = 8
MROWS = R // NCORES      # 1024 rows per core
P = 128                  # SBUF partitions
NT_ROW = R // P          # 64 row tiles of (128, 256)
MB = MROWS // P          # 8 m-blocks per core
QUAD = 2048              # psum quad width (4 banks)
NQ = R // QUAD           # 4 quads per block row
NG = 4                   # preprocessing groups (2048 rows each)
TPG = NT_ROW // NG       # 16 row tiles per group
GR = R // NG             # 2048 rows per group
FP8_SCALE = 16.0         # rows scaled to ~16x unit norm before fp8 quantize
SIM_SCALE = FP8_SCALE * FP8_SCALE
BIG = 30000.0            # diag mask subtrahend (|sim_scaled| <= ~300)
NEG_INIT = -1.0e30

_CACHE = {}


def _host_constants():
    ident = np.eye(P, dtype=np.float32)
    bigI = (np.eye(P) * BIG).astype(np.float32)
    return {"ident_f": ident, "bigI": bigI}


def _build_nc():
    from contextlib import ExitStack

    import concourse.bass as bass
    import concourse.mybir as mybir
    import concourse.tile as tile
    from concourse import bacc

    f32 = mybir.dt.float32
    bf16 = mybir.dt.bfloat16
    f8 = mybir.dt.float8e4
    u16 = mybir.dt.uint16
    AF = mybir.ActivationFunctionType
    ALU = mybir.AluOpType
    X = mybir.AxisListType.X
    DR = mybir.MatmulPerfMode.DoubleRow

    nc = bacc.Bacc(
        "TRN2",
        target_bir_lowering=False,
        debug=False,
        enable_asserts=False,
        num_devices=NCORES,
    )

    # z is passed from the host as bf16 pairs: [row, d, 2] where [..., 1] is
    # the high half of the f32 word (bf16 truncation).
    z_dram = nc.dram_tensor("z", [R, D, 2], bf16, kind="ExternalInput")
    ident_dram = nc.dram_tensor("ident_f", [P, P], f32, kind="ExternalInput")
    bigI_dram = nc.dram_tensor("bigI", [P, P], f32, kind="ExternalInput")
    rowmax_dram = nc.dram_tensor("row_max", [P, MB], f32, kind="ExternalOutput")
    pos_dram = nc.dram_tensor("pos", [P, MB], f32, kind="ExternalOutput")
    z8_dram = nc.dram_tensor("z8_scratch", [R, D // 2], u16, kind="Internal")

    with tile.TileContext(nc) as tc, ExitStack() as ctx:
        singles = ctx.enter_context(tc.tile_pool(name="singles", bufs=1))
        big = ctx.enter_context(tc.tile_pool(name="big", bufs=1))
        zb_pool = ctx.enter_context(tc.tile_pool(name="zb_pool", bufs=2))
        z8_pool = ctx.enter_context(tc.tile_pool(name="z8_pool", bufs=2))
        cand_pool = ctx.enter_context(tc.tile_pool(name="cand_pool", bufs=3))
        scr_pool = ctx.enter_context(tc.tile_pool(name="scr_pool", bufs=1))
        psum = ctx.enter_context(
            tc.tile_pool(name="psum", bufs=2, space=bass.MemorySpace.PSUM)
        )

        # --- constants ---
        ident_f = singles.tile([P, P], f32)
        nc.sync.dma_start(out=ident_f, in_=ident_dram.ap())
        bigI = singles.tile([P, P], f32)
        nc.sync.dma_start(out=bigI, in_=bigI_dram.ap())

        # --- persistent buffers ---
        zT16 = big.tile([P, R], u16)            # transposed fp8 pairs (scratch)
        zT8p = big.tile([P, 2, R], f8)          # de-interleaved k-tile planes
        n2 = singles.tile([P, NT_ROW], f32)
        inv16 = singles.tile([P, NT_ROW], f32)
        rmq = singles.tile([P, MB, NQ], f32)    # per-quad row maxes
        rowmax_sb = singles.tile([P, MB], f32)
        pos_sb = singles.tile([P, MB], f32)

        # scratch (write-only outputs of reduce ops, engine-ordered reuse)
        sq_scr = scr_pool.tile([P, D], bf16)
        sq_scr_v = scr_pool.tile([P, D], bf16)
        scrPos = scr_pool.tile([P, P], f32)

        # fp8-pair byte view of zT16: [p, i, col] with d = 2p + i. Codegen
        # requires contiguous per-k-tile columns for DoubleRow, so Pool
        # de-interleaves this into zT8p after each group's transpose.
        zt16v = zT16[:, :].bitcast(f8).rearrange("p (r i) -> p i r", i=2)
        zt8 = zT8p

        # z load view: [p, d, t, two]; [..., 1] selects the bf16 truncation.
        # (d-major staging keeps the strided pair-select DMA 3-dim balanceable)
        z_src = z_dram.ap().rearrange("(t p) d two -> p d t two", p=P)

        zbs = [None] * NG
        z8s = [None] * NG

        # issue all group loads up front (SP queue)
        for g in range(NG):
            zb = zb_pool.tile([P, D, TPG], bf16, name="zb")
            nc.sync.dma_start(
                out=zb, in_=z_src[:, :, g * TPG : (g + 1) * TPG, 1]
            )
            zbs[g] = zb

        def preproc_norms(g, sub, nsub):
            """norms for tiles [sub*TPG/nsub, (sub+1)*TPG/nsub) of group g"""
            tps = TPG // nsub
            for j in range(sub * tps, (sub + 1) * tps):
                t = g * TPG + j
                if j % 8 < 5:
                    nc.scalar.activation(
                        out=sq_scr,
                        in_=zbs[g][:, :, j],
                        func=AF.Square,
                        accum_out=n2[:, t : t + 1],
                    )
                else:
                    nc.vector.scalar_tensor_tensor(
                        out=sq_scr_v,
                        in0=zbs[g][:, :, j],
                        scalar=1.0,
                        in1=zbs[g][:, :, j],
                        op0=ALU.mult,
                        op1=ALU.mult,
                        accum_out=n2[:, t : t + 1],
                    )

        def preproc_finish_norms(g):
            gs = slice(g * TPG, (g + 1) * TPG)
            # nrm/16 = sqrt(n2/256); inv16 = 16/||z||
            nc.scalar.activation(
                out=inv16[:, gs], in_=n2[:, gs], func=AF.Sqrt, scale=1.0 / 256.0
            )
            nc.vector.reciprocal(out=inv16[:, gs], in_=inv16[:, gs])

        def preproc_quant(g, sub, nsub):
            tps = TPG // nsub
            if sub == 0:
                z8s[g] = z8_pool.tile([P, TPG, D], f8, name="z8")
            for j in range(sub * tps, (sub + 1) * tps):
                t = g * TPG + j
                nc.gpsimd.tensor_scalar_mul(
                    z8s[g][:, j, :], zbs[g][:, :, j], inv16[:, t : t + 1]
                )

        def preproc_store_transpose(g):
            r0 = g * GR
            nc.sync.dma_start(
                out=z8_dram.ap()
                .bitcast(f8)
                .rearrange("(t p) d -> p t d", p=P)[:, g * TPG : (g + 1) * TPG, :],
                in_=z8s[g][:],
            )
            nc.sync.dma_start(
                out=zT16[:, r0 : r0 + GR],
                in_=z8_dram.ap()[r0 : r0 + GR, :],
                transpose=True,
            )
            for i in range(2):
                nc.gpsimd.tensor_scalar_mul(
                    zT8p[:, i, r0 : r0 + GR], zt16v[:, i, r0 : r0 + GR], 1.0
                )

        def preproc_group(g):
            for s in range(2):
                preproc_norms(g, s, 2)
            preproc_finish_norms(g)
            for s in range(2):
                preproc_quant(g, s, 2)
            preproc_store_transpose(g)

        def emit_quad(q, b):
            o = b * P
            pp = psum.tile([P, QUAD], f32, name="pp")
            for u in range(QUAD // 512):
                col = q * QUAD + u * 512
                nc.tensor.matmul(
                    pp[:, u * 512 : (u + 1) * 512],
                    zt8[:, :, o : o + P],
                    zt8[:, :, col : col + 512],
                    start=True,
                    stop=True,
                    perf_mode=DR,
                )
            if q == 0:
                # mask self-similarity (diag block at cols o..o+128)
                nc.vector.tensor_sub(pp[:, o : o + P], pp[:, o : o + P], bigI)
            if q == 2:
                # positives: diag of the block at columns 4096+o
                nc.vector.scalar_tensor_tensor(
                    out=scrPos,
                    in0=pp[:, o : o + P],
                    scalar=1.0,
                    in1=ident_f,
                    op0=ALU.mult,
                    op1=ALU.mult,
                    accum_out=pos_sb[:, b : b + 1],
                )
            slot = rmq[:, b, q : q + 1]
            if b not in (1, 4, 6):
                # ACT evacuates the quad to bf16, DVE 4x tail reduce
                cand = cand_pool.tile([P, QUAD], bf16, name="cand")
                nc.scalar.copy(out=cand, in_=pp[:])
                nc.vector.reduce_max(out=slot, in_=cand, axis=X)
            else:
                # DVE reduces the quad straight off psum
                nc.vector.reduce_max(out=slot, in_=pp[:], axis=X)

        # --- pipeline emission ---
        preproc_group(0)
        preproc_group(1)
        for q in range(NQ):
            for b in range(MB):
                emit_quad(q, b)
                if q < 2:
                    g = q + 2
                    if b == 1:
                        preproc_norms(g, 0, 2)
                    elif b == 3:
                        preproc_norms(g, 1, 2)
                        preproc_finish_norms(g)
                    elif b == 5:
                        preproc_quant(g, 0, 2)
                    elif b == 7:
                        preproc_quant(g, 1, 2)
                        preproc_store_transpose(g)

        nc.vector.reduce_max(out=rowmax_sb, in_=rmq[:, :, :], axis=X)
        nc.sync.dma_start(out=rowmax_dram.ap(), in_=rowmax_sb[:])
        nc.sync.dma_start(out=pos_dram.ap(), in_=pos_sb[:])

    nc.compile()
    return nc


def _get_nc():
    if "nc" not in _CACHE:
        _CACHE["nc"] = _build_nc()
    return _CACHE["nc"]


def _finish(rowmax_all: np.ndarray, pos_all: np.ndarray) -> np.ndarray:
    scale = 1.0 / (SIM_SCALE * TEMPERATURE)
    negmax = rowmax_all.astype(np.float64) * scale
    pos = pos_all.astype(np.float64) * scale
    m = pos.max()
    lse = np.log(np.exp(pos - m).sum()) + m
    return np.array(negmax.mean() - lse, dtype=np.float32)


def kernel(z_i: np.ndarray, z_j: np.ndarray, _collect=None, _run_kwargs=None) -> np.ndarray:
    import ml_dtypes
    from concourse.bass_utils import run_bass_kernel_spmd

    z_full = np.concatenate(
        [np.asarray(z_i, np.float32), np.asarray(z_j, np.float32)], axis=0
    )
    consts = _host_constants()
    in_maps = []
    for k in range(NCORES):
        zk = np.ascontiguousarray(np.roll(z_full, -k * MROWS, axis=0))
        zk = zk.view(ml_dtypes.bfloat16).reshape(R, D, 2)
        in_maps.append({"z": zk, **consts})
    nc = _get_nc()
    res = run_bass_kernel_spmd(
        nc, in_maps, core_ids=list(range(NCORES)), **(_run_kwargs or {})
    )
    if _collect is not None:
        _collect.append(res)
    rowmax_all = np.concatenate(
        [r["row_max"].T.reshape(-1) for r in res.results]
    )  # (8192,) in original row order
    pos_all = np.concatenate([r["pos"].T.reshape(-1) for r in res.results])
    return _finish(rowmax_all, pos_all)
